# revision 4
# baseline (speedup 1.0000x reference)
"""Trainium2 Bass kernel for nn_AttentionBlock (B=4, T=2048, C=1024, H=16,
SwiGLU hidden 2730), distributed over 8 NeuronCores.

Sharding: data-parallel over (batch, query-half) with a block permutation that
makes the causal workload uniform across cores. Core c = 2*b + h owns query
512-blocks {0,3} (h=0) or {1,2} (h=1) of batch b. The host permutes the
sequence at 512-block granularity (h=0: [1,0,2,3], h=1: [0,1,3,2]) so that on
EVERY core the owned query blocks sit at permuted slots {1,3}. Causal masking
between permuted blocks is supplied as per-core mask data (ones / zeros /
128-diagonal triangles), so a single SPMD program serves both core types:
unit A (slot 1) runs 4 key-pairs, unit B (slot 3) runs 8 key-pairs, of which
pairs 0-3 are causally full for both core types (no mask multiply).

Precision: K and Q projections run fp8e4 DoubleRow (weights x16, h1 quantized
to fp8); q/k are kept fp8 (x16) for the score matmuls; exp folds the 1/2048
descale. V, attention-value, proj, and the whole MLP stay fp16 (fp8 there
fails the error budget: early tokens have no softmax averaging to wash out v
error, and mlp_out is ~6x larger than attn_out). Softmax denominators use the
single-pass DVE reciprocal_approx_fast instead of the slow InstReciprocal.

MLP work (both layers) is emitted interleaved into the attention head loop as
PE filler so the tensor engine stays busy while the scalar engine chews
through the exp() stream.
"""

import numpy as np
import ml_dtypes

import concourse.bacc as bacc
import concourse.mybir as mybir
import concourse.tile as tile
from concourse.bass_utils import run_bass_kernel_spmd

P = 128
C = 1024            # d_model
T = 2048            # sequence length
NQ = 1024           # query tokens per core
H = 16              # heads
HD = 64             # head dim
HID = 2730          # SwiGLU hidden
HIDP = 2816         # padded hidden (22 * 128)
KC = C // P         # 8 contraction chunks of 128
HT = HIDP // P      # 22 hidden tiles
EPS = 1e-6
WS = 16.0           # fp8 weight scale for wq/wk
E4NP = ml_dtypes.float8_e4m3

f32 = mybir.dt.float32
fp16 = mybir.dt.float16
f8 = mybir.dt.float8e4

_NC_CACHE = {}


def _build():
    if "nc" in _NC_CACHE:
        return _NC_CACHE["nc"]
    nc = bacc.Bacc()

    xf = nc.declare_dram_parameter("xf", [P, 4, KC, 512], fp16, False)
    xq = nc.declare_dram_parameter("xq", [P, 8, C], fp16, False)
    wq8 = nc.declare_dram_parameter("wq8", [P, 2, KC, 512], f8, False)
    wk8 = nc.declare_dram_parameter("wk8", [P, 2, KC, 512], f8, False)
    wv = nc.declare_dram_parameter("wv", [P, 2, KC, 512], fp16, False)
    wp = nc.declare_dram_parameter("wp", [P, 2, KC, 512], fp16, False)
    w1p = nc.declare_dram_parameter("w1p", [P, HT, KC, P], fp16, False)
    w2p = nc.declare_dram_parameter("w2p", [P, HT, KC, P], fp16, False)
    w3p = nc.declare_dram_parameter("w3p", [P, 4, HT, 256], fp16, False)
    trit = nc.declare_dram_parameter("trit", [2, P, 1024], fp16, False)
    flg = nc.declare_dram_parameter("flg", [P, 8], f32, False)
    vones = nc.declare_dram_parameter("vones", [P, 16, 16], fp16, False)
    ones16 = nc.declare_dram_parameter("ones16", [P, 1], fp16, False)
    out = nc.declare_dram_parameter("out", [NQ, C], f32, True)

    Exp = mybir.ActivationFunctionType.Exp
    Sqrt = mybir.ActivationFunctionType.Sqrt
    Tanh = mybir.ActivationFunctionType.Tanh
    mult = mybir.AluOpType.mult
    add = mybir.AluOpType.add
    DR = mybir.MatmulPerfMode.DoubleRow

    with tile.TileContext(nc, pool_alloc_mode="queue") as tc:
        with tc.tile_pool(name="base", bufs=1) as base:
            h16own = base.tile([P, KC, NQ], fp16)     # rmsnorm(x)^T, own slots
            k8 = base.tile([P, KC, T], f8)            # K^T x16, fp8 (2MB)
            q8 = base.tile([P, KC, NQ], f8)           # Q^T x16, fp8 (1MB)
            v_sb = base.tile([P, 16, 16, 65], fp16)   # V + ones col (4.26MB)
            y16 = base.tile([P, KC, NQ], fp16)        # attn out, feature-major
            acc = base.tile([P, NQ // P, C], fp16)    # x + attn + mlp
            u_sb = base.tile([P, HT, 512], fp16)      # h@w2 then u, per j
            a_sb = base.tile([P, HT, 512], fp16)      # h@w1 staging, per j
            trit_sb = base.tile([P, 2, 1024], fp16)   # diag triangle masks
            flg_sb = base.tile([P, 8], f32)           # ones/zeros pair flags
            ones_sb = base.tile([P, 1], fp16)
            eps_sb = base.tile([1, 1], f32)
            nc.gpsimd.memset(eps_sb[:], EPS)
            nc.sync.dma_start(trit_sb[:], trit.rearrange("m p q -> p m q"))
            nc.sync.dma_start(flg_sb[:], flg[:])
            nc.sync.dma_start(ones_sb[:], ones16[:])
            nc.sync.dma_start(v_sb[:, :, :, 64], vones[:])

            # ---------------- Phase 0+1: rmsnorm, then qkv ----------------
            with tc.tile_pool(name="ph8", bufs=1) as ph8:
                h8 = ph8.tile([P, KC, T], f8)         # rmsnorm(x)^T fp8 (2MB)
                h16oth = ph8.tile([P, KC, NQ], fp16)  # other cores' slots (0,2)
                with tc.tile_pool(name="ph0x", bufs=1) as ph0x, \
                     tc.tile_pool(name="ph0t", bufs=2) as ph0t, \
                     tc.tile_pool(name="ps0", bufs=2, space="PSUM") as ps0:
                    def hsl(tb):
                        dst = h16own if tb % 2 else h16oth
                        c0 = (tb // 2) * 512
                        return dst[:, :, c0:c0 + 512]
                    for tb in range(T // 512):
                        nc.sync.dma_start(hsl(tb), xf[:, tb])
                    with nc.named_scope("rmsnorm"):
                        for tb in range(T // 512):
                            hs = hsl(tb)
                            x2 = ph0x.tile([P, KC, 512], fp16, tag="x2")
                            nc.vector.tensor_tensor(x2[:], hs, hs, mult)
                            ssq = ps0.tile([1, 512], f32, tag="ssq")
                            for kc in range(KC):
                                nc.tensor.matmul(
                                    ssq[:], lhsT=ones_sb[:], rhs=x2[:, kc],
                                    start=(kc == 0), stop=(kc == KC - 1))
                            rms = ph0t.tile([1, 512], f32, tag="rms")
                            nc.scalar.activation(rms[:], ssq[:], Sqrt,
                                                 bias=eps_sb[0:1, :], scale=1.0 / C)
                            rinv = ph0t.tile([1, 512], f32, tag="rinv")
                            nc.vector.reciprocal_approx_fast(rinv[:], rms[:])
                            r16 = ph0t.tile([1, 512], fp16, tag="r16")
                            nc.vector.tensor_copy(r16[:], rinv[:])
                            s_bc = ph0t.tile([P, 512], fp16, tag="sbc")
                            nc.gpsimd.partition_broadcast(s_bc[:], r16[0:1, :])
                            nc.vector.tensor_tensor(
                                hs, hs,
                                s_bc[:, None, :].to_broadcast((P, KC, 512)), mult)
                            nc.vector.tensor_copy(
                                h8[:, :, tb * 512:(tb + 1) * 512], hs)

                # qkv: K/Q in fp8 DoubleRow, V in fp16
                with tc.tile_pool(name="wq8p", bufs=2) as wq8p, \
                     tc.tile_pool(name="wv16p", bufs=1) as wv16p, \
                     tc.tile_pool(name="ps1", bufs=4, space="PSUM") as ps1:
                    with nc.named_scope("qkv"):
                        for half in range(2):
                            wk_c = wq8p.tile([P, KC, 512], f8, tag="wc")
                            nc.sync.dma_start(wk_c[:], wk8[:, half])
                            for oi in range(4):
                                ot = half * 4 + oi
                                for tb in range(4):
                                    ps = ps1.tile([P, 512], f32, tag="mm")
                                    for pr in range(KC // 2):
                                        nc.tensor.matmul(
                                            ps[:],
                                            lhsT=wk_c[:, 2 * pr:2 * pr + 2,
                                                      oi * P:(oi + 1) * P],
                                            rhs=h8[:, 2 * pr:2 * pr + 2,
                                                   tb * 512:(tb + 1) * 512],
                                            start=(pr == 0), stop=(pr == 3),
                                            perf_mode=DR)
                                    nc.vector.tensor_copy(
                                        k8[:, ot, tb * 512:(tb + 1) * 512], ps[:])
                        for half in range(2):
                            wq_c = wq8p.tile([P, KC, 512], f8, tag="wc")
                            nc.sync.dma_start(wq_c[:], wq8[:, half])
                            for oi in range(4):
                                ot = half * 4 + oi
                                for j, t0 in enumerate((512, 1536)):
                                    ps = ps1.tile([P, 512], f32, tag="mm")
                                    for pr in range(KC // 2):
                                        nc.tensor.matmul(
                                            ps[:],
                                            lhsT=wq_c[:, 2 * pr:2 * pr + 2,
                                                      oi * P:(oi + 1) * P],
                                            rhs=h8[:, 2 * pr:2 * pr + 2,
                                                   t0:t0 + 512],
                                            start=(pr == 0), stop=(pr == 3),
                                            perf_mode=DR)
                                    nc.vector.tensor_copy(
                                        q8[:, ot, j * 512:(j + 1) * 512], ps[:])
                        for vf in range(2):
                            wv_c = wv16p.tile([P, KC, 512], fp16, tag="wc")
                            nc.sync.dma_start(wv_c[:], wv[:, vf])
                            for kt in range(16):
                                slot, sub = kt // 4, kt % 4
                                src = h16own if slot % 2 else h16oth
                                c0 = ((slot // 2) * 4 + sub) * P
                                ps = ps1.tile([P, 512], f32, tag="mm")
                                for kc in range(KC):
                                    nc.tensor.matmul(
                                        ps[:], lhsT=src[:, kc, c0:c0 + P],
                                        rhs=wv_c[:, kc, :],
                                        start=(kc == 0), stop=(kc == KC - 1))
                                nc.vector.tensor_copy(
                                    v_sb[:, kt, 8 * vf:8 * (vf + 1), 0:64],
                                    ps[:].rearrange("p (h d) -> p h d", d=64))

            # ---------------- Phase 2: attn + interleaved MLP ----------------
            with tc.tile_pool(name="att", bufs=3) as att, \
                 tc.tile_pool(name="attr", bufs=3) as attr, \
                 tc.tile_pool(name="w12", bufs=2) as w12, \
                 tc.tile_pool(name="w3pool", bufs=1) as w3pool, \
                 tc.tile_pool(name="silu", bufs=2) as silp, \
                 tc.tile_pool(name="ps2s", bufs=2, space="PSUM") as ps2s, \
                 tc.tile_pool(name="ps2y", bufs=2, space="PSUM") as ps2y, \
                 tc.tile_pool(name="psml", bufs=2, space="PSUM") as psml:
                nc.sync.dma_start(acc[:], xq[:])

                # ---- filler generator: mlp_in / mlp_out chunks ----
                # silu is flushed in one burst per j-block so the scalar
                # engine's activation table doesn't thrash between Exp/Swish.
                def filler_gen():
                    # mlp_in for both j-blocks: a/b staged raw to SBUF; silu,
                    # mult and mlp_out run post-attn (keeps the scalar
                    # engine's exp stream free of table switches).
                    for j, t0 in enumerate((0, 512)):
                        tsl = slice(t0, t0 + 512)
                        for ht in range(HT):
                            w1c = w12.tile([P, KC, P], fp16, tag="w1c")
                            w2c = w12.tile([P, KC, P], fp16, tag="w2c")
                            nc.sync.dma_start(w1c[:], w1p[:, ht])
                            nc.sync.dma_start(w2c[:], w2p[:, ht])
                            ps_a = psml.tile([P, 512], f32, tag="mm")
                            for kc in range(KC):
                                nc.tensor.matmul(
                                    ps_a[:], lhsT=w1c[:, kc],
                                    rhs=h16own[:, kc, tsl],
                                    start=(kc == 0), stop=(kc == KC - 1))
                            nc.vector.tensor_copy(a_sb[:, ht, :], ps_a[:])
                            yield
                            ps_b = psml.tile([P, 512], f32, tag="mm")
                            for kc in range(KC):
                                nc.tensor.matmul(
                                    ps_b[:], lhsT=w2c[:, kc],
                                    rhs=h16own[:, kc, tsl],
                                    start=(kc == 0), stop=(kc == KC - 1))
                            nc.vector.tensor_copy(u_sb[:, ht, :], ps_b[:])
                            yield
                        for ht in range(HT):
                            th = silp.tile([P, 512], fp16, tag="th")
                            nc.scalar.activation(th[:], a_sb[:, ht, :], Tanh,
                                                 scale=0.5)
                            sg = silp.tile([P, 512], fp16, tag="sg")
                            nc.vector.tensor_scalar(
                                sg[:], th[:], 1.0, 0.5, add, mult)
                            nc.vector.tensor_tensor(
                                u_sb[:, ht, :], a_sb[:, ht, :],
                                u_sb[:, ht, :], mult)
                            nc.vector.tensor_tensor(
                                u_sb[:, ht, :], sg[:], u_sb[:, ht, :], mult)
                        yield
                        for ofq in range(4):
                            w3c = w3pool.tile([P, HT, 256], fp16, tag="w3c")
                            nc.sync.dma_start(w3c[:], w3p[:, ofq])
                            for qt in range(4):
                                ps = psml.tile([P, 512], f32, tag="mm")
                                for ht in range(HT):
                                    nc.tensor.matmul(
                                        ps[:, 0:256],
                                        lhsT=u_sb[:, ht, qt * P:(qt + 1) * P],
                                        rhs=w3c[:, ht, :],
                                        start=(ht == 0), stop=(ht == HT - 1))
                                asl = acc[:, 4 * j + qt, ofq * 256:(ofq + 1) * 256]
                                nc.vector.tensor_tensor(asl, asl, ps[:, 0:256], add)
                                yield

                fill = filler_gen()
                done = [False]
                head_budget = [99]

                def F(n=1):
                    for _ in range(n):
                        if not done[0] and head_budget[0] > 0:
                            head_budget[0] -= 1
                            try:
                                next(fill)
                            except StopIteration:
                                done[0] = True

                # ---- attention head loop ----
                with nc.named_scope("attn"):
                    for h in range(H):
                        head_budget[0] = 6
                        hp = 64 * (h % 2)
                        ho = h // 2
                        for j, (qofs, npair) in enumerate(((0, 4), (512, 8))):
                            qsl = slice(qofs, qofs + 512)
                            ps_y = ps2y.tile([65, 512], f32, tag="y")
                            p_tiles = {}

                            def emit_scores(pr, j=j, qsl=qsl, hp=hp, ho=ho,
                                            p_tiles=p_tiles):
                                ps_s = ps2s.tile([P, 1024], f32, tag="s")
                                for half in range(2):
                                    kb = pr * 2 + half
                                    nc.tensor.matmul(
                                        ps_s[:, half * 512:(half + 1) * 512],
                                        lhsT=k8[hp:hp + 64, ho, kb * P:(kb + 1) * P],
                                        rhs=q8[hp:hp + 64, ho, qsl],
                                        start=True, stop=True)
                                p_sb = att.tile([P, 1024], fp16, tag="p")
                                nc.scalar.activation(p_sb[:], ps_s[:], Exp,
                                                     scale=1.0 / 2048.0)
                                # masks: first 2 pairs of each unit are
                                # ones-or-zeros (per-core flag), last 2 are the
                                # shared 128-diagonal triangles; unit B pairs
                                # 0-3 are causally full for both core types.
                                if (j == 0 and pr < 2) or (j == 1 and pr in (4, 5)):
                                    nc.gpsimd.tensor_scalar_mul(
                                        p_sb[:], p_sb[:], flg_sb[:, pr:pr + 1])
                                elif (j == 0 and pr >= 2) or (j == 1 and pr >= 6):
                                    m = pr - 2 if j == 0 else pr - 6
                                    nc.gpsimd.tensor_tensor(
                                        p_sb[:], p_sb[:], trit_sb[:, m, :], mult)
                                p_tiles[pr] = p_sb

                            def emit_av(pr, npair=npair, h=h, ps_y=ps_y,
                                        p_tiles=p_tiles):
                                p_sb = p_tiles.pop(pr)
                                for half in range(2):
                                    kb = pr * 2 + half
                                    nc.tensor.matmul(
                                        ps_y[:],
                                        lhsT=v_sb[:, kb, h, :],
                                        rhs=p_sb[:, half * 512:(half + 1) * 512],
                                        start=(kb == 0), stop=(kb == 2 * npair - 1))

                            for pr in range(npair):
                                emit_scores(pr)
                                F(1)
                                if pr >= 1:
                                    emit_av(pr - 1)
                            emit_av(npair - 1)
                            yu = attr.tile([64, 512], fp16, tag="yu")
                            nc.vector.tensor_copy(yu[:], ps_y[0:64, :])
                            den = attr.tile([1, 512], f32, tag="den")
                            nc.vector.tensor_copy(den[:], ps_y[64:65, :])
                            rinv = attr.tile([1, 512], f32, tag="rd")
                            nc.vector.reciprocal_approx_fast(rinv[:], den[:])
                            r_bc = attr.tile([64, 512], f32, tag="rbc")
                            nc.gpsimd.partition_broadcast(r_bc[:], rinv[0:1, :])
                            nc.vector.tensor_tensor(
                                y16[hp:hp + 64, ho, j * 512:(j + 1) * 512],
                                yu[:], r_bc[:], mult)
                            F(2)

                # drain remaining filler work
                head_budget[0] = 999
                F(200)

            # ---- proj (attn pools closed; own scope) ----
            with tc.tile_pool(name="projw", bufs=2) as projw, \
                 tc.tile_pool(name="ps3", bufs=2, space="PSUM") as ps3:
                with nc.named_scope("proj"):
                    for of in range(2):
                        wp_c = projw.tile([P, KC, 512], fp16, tag="wpc")
                        nc.sync.dma_start(wp_c[:], wp[:, of])
                        for qt in range(NQ // P):
                            ps = ps3.tile([P, 512], f32, tag="mm")
                            for kc in range(KC):
                                nc.tensor.matmul(
                                    ps[:],
                                    lhsT=y16[:, kc, qt * P:(qt + 1) * P],
                                    rhs=wp_c[:, kc, :],
                                    start=(kc == 0), stop=(kc == KC - 1))
                            asl = acc[:, qt, of * 512:(of + 1) * 512]
                            nc.vector.tensor_tensor(asl, asl, ps[:], add)

            # ---- fp16 acc -> f32 out ----
            with tc.tile_pool(name="stg", bufs=2) as stg:
                outv = out.rearrange("(qt p) f -> p qt f", p=P)
                for qt in range(NQ // P):
                    st = stg.tile([P, C], f32, tag="st")
                    nc.vector.tensor_copy(st[:], acc[:, qt, :])
                    nc.sync.dma_start(outv[:, qt, :], st[:])

    nc.finalize()
    _NC_CACHE["nc"] = nc
    return nc


def _prep_inputs(x, w_attn, w_proj, w1, w2, w3, g1, g2):
    """Host-side preprocessing -> list of 8 per-core input maps."""
    x = np.asarray(x, np.float32)
    w_attn = np.asarray(w_attn, np.float32)
    g1 = np.asarray(g1, np.float32)
    g2 = np.asarray(g2, np.float32)

    def to8(w):
        return np.clip(w * WS, -240, 240).astype(E4NP)

    def whalf(w):
        # [C, C] -> [P, half, KC, 512] contiguous per-partition DMA layout
        return np.ascontiguousarray(
            w.reshape(KC, P, 2, 512).transpose(1, 2, 0, 3))

    wq8 = whalf(to8(g1[:, None] * w_attn[:, 0:C]))
    wk8 = whalf(to8(g1[:, None] * w_attn[:, C:2 * C]))
    wvh = whalf((g1[:, None] * w_attn[:, 2 * C:3 * C]).astype(np.float16))
    wph = whalf(np.asarray(w_proj, np.float32).astype(np.float16))
    w1p = np.zeros((C, HIDP), np.float16)
    w1p[:, :HID] = (g2[:, None] * np.asarray(w1, np.float32)).astype(np.float16)
    w2p = np.zeros((C, HIDP), np.float16)
    w2p[:, :HID] = (g2[:, None] * np.asarray(w2, np.float32)).astype(np.float16)
    # [C, HIDP] -> [P, HT, KC, P]
    w1p = np.ascontiguousarray(w1p.reshape(KC, P, HT, P).transpose(1, 2, 0, 3))
    w2p = np.ascontiguousarray(w2p.reshape(KC, P, HT, P).transpose(1, 2, 0, 3))
    w3f = np.zeros((HIDP, C), np.float16)
    w3f[:HID, :] = np.asarray(w3, np.float32).astype(np.float16)
    # [HIDP, C] -> [P, 4, HT, 256]
    w3p = np.ascontiguousarray(w3f.reshape(HT, P, 4, 256).transpose(1, 2, 0, 3))

    # triangle masks: tri[m][i, q] = 1 if i + 128*m <= q  (q in 0..511);
    # trit entry m = [tri(2m) | tri(2m+1)] covering one kb pair.
    ii = np.arange(P)[:, None]
    qq = np.arange(512)[None, :]
    tri = [(ii + P * m <= qq).astype(np.float16) for m in range(4)]
    trit = np.stack([np.concatenate([tri[0], tri[1]], axis=1),
                     np.concatenate([tri[2], tri[3]], axis=1)], axis=0)

    vones = np.ones((P, 16, 16), np.float16)
    ones16 = np.ones((P, 1), np.float16)

    perms = {0: [1, 0, 2, 3], 1: [0, 1, 3, 2]}

    in_maps = []
    for core in range(8):
        b, h = core // 2, core % 2
        pi = perms[h]
        xp = np.concatenate([x[b, g * 512:(g + 1) * 512] for g in pi], axis=0)
        # x^T [C, T] -> [P, tb, KC, 512]
        xt = xp.T.astype(np.float16).reshape(KC, P, 4, 512)
        xf = np.ascontiguousarray(xt.transpose(1, 2, 0, 3))
        xqr = np.concatenate([xp[512:1024], xp[1536:2048]], axis=0).astype(np.float16)
        xq = np.ascontiguousarray(xqr.reshape(8, P, C).transpose(1, 0, 2))

        # pair flags: unit A (q block pi[1]) pairs 0-1 cover k slot 0
        # (block pi[0]); unit B (q block pi[3]) pairs 4-5 cover k slot 2
        # (block pi[2]). 1.0 = causally full, 0.0 = fully masked.
        flg = np.ones((P, 8), np.float32)
        flg[:, 0] = flg[:, 1] = 1.0 if pi[0] < pi[1] else 0.0
        flg[:, 4] = flg[:, 5] = 1.0 if pi[2] < pi[3] else 0.0

        in_maps.append({
            "xf": xf, "xq": xq, "wq8": wq8, "wk8": wk8, "wv": wvh, "wp": wph,
            "w1p": w1p, "w2p": w2p, "w3p": w3p, "trit": trit, "flg": flg,
            "vones": vones, "ones16": ones16,
        })
    return in_maps


def _run(inputs, trace=False):
    nc = _build()
    in_maps = _prep_inputs(**inputs)
    res = run_bass_kernel_spmd(
        nc, in_maps, core_ids=list(range(8)), trace=trace,
        trace_cores=list(range(8)) if trace else None)
    B = 4
    perms = {0: [1, 0, 2, 3], 1: [0, 1, 3, 2]}
    out = np.empty((B, T, C), np.float32)
    for core in range(8):
        b, h = core // 2, core % 2
        pi = perms[h]
        r = res.results[core]["out"]
        out[b, pi[1] * 512:(pi[1] + 1) * 512] = r[0:512]
        out[b, pi[3] * 512:(pi[3] + 1) * 512] = r[512:1024]
    return out, res


def kernel(**inputs):
    out, _ = _run(inputs, trace=False)
    return out


# revision 5
# speedup vs baseline: 2.7788x; 2.7788x over previous
"""Trainium2 Bass kernel for nn_AttentionBlock (B=4, T=2048, C=1024, H=16,
SwiGLU hidden 2730), distributed over 8 NeuronCores.

Sharding: data-parallel over (batch, query-half) with a block permutation that
makes the causal workload uniform across cores. Core c = 2*b + h owns query
512-blocks {0,3} (h=0) or {1,2} (h=1) of batch b. The host permutes the
sequence at 512-block granularity (h=0: [1,0,2,3], h=1: [0,1,3,2]) so that on
EVERY core the owned query blocks sit at permuted slots {1,3}. Causal masking
between permuted blocks is supplied as per-core mask data (ones / zeros /
128-diagonal triangles), so a single SPMD program serves both core types:
unit A (slot 1) runs 4 key-pairs, unit B (slot 3) runs 8 key-pairs, of which
pairs 0-3 are causally full for both core types (no mask multiply).

Precision: K and Q projections run fp8e4 DoubleRow (weights x16, h1 quantized
to fp8); q/k are kept fp8 (x16) for the score matmuls; exp folds the 1/2048
descale. V, attention-value, proj, and the whole MLP stay fp16 (fp8 there
fails the error budget: early tokens have no softmax averaging to wash out v
error, and mlp_out is ~6x larger than attn_out). Softmax denominators use the
single-pass DVE reciprocal_approx_fast instead of the slow InstReciprocal.

MLP work (both layers) is emitted interleaved into the attention head loop as
PE filler so the tensor engine stays busy while the scalar engine chews
through the exp() stream.
"""

import numpy as np
import ml_dtypes

import concourse.bacc as bacc
import concourse.mybir as mybir
import concourse.tile as tile
from concourse.bass_utils import run_bass_kernel_spmd

P = 128
C = 1024            # d_model
T = 2048            # sequence length
NQ = 1024           # query tokens per core
H = 16              # heads
HD = 64             # head dim
HID = 2730          # SwiGLU hidden
HIDP = 2816         # padded hidden (22 * 128)
KC = C // P         # 8 contraction chunks of 128
HT = HIDP // P      # 22 hidden tiles
EPS = 1e-6
WS = 16.0           # fp8 weight scale for wq/wk
E4NP = ml_dtypes.float8_e4m3

f32 = mybir.dt.float32
fp16 = mybir.dt.float16
f8 = mybir.dt.float8e4

_NC_CACHE = {}


def _build():
    if "nc" in _NC_CACHE:
        return _NC_CACHE["nc"]
    nc = bacc.Bacc()

    xf = nc.declare_dram_parameter("xf", [P, 4, KC, 512], fp16, False)
    xq = nc.declare_dram_parameter("xq", [P, 8, C], fp16, False)
    wq8 = nc.declare_dram_parameter("wq8", [P, 2, KC, 512], f8, False)
    wk8 = nc.declare_dram_parameter("wk8", [P, 2, KC, 512], f8, False)
    wv = nc.declare_dram_parameter("wv", [P, 2, KC, 512], fp16, False)
    wp = nc.declare_dram_parameter("wp", [P, 2, KC, 512], fp16, False)
    w1p = nc.declare_dram_parameter("w1p", [P, HT, KC, P], fp16, False)
    w2p = nc.declare_dram_parameter("w2p", [P, HT, KC, P], fp16, False)
    w3p = nc.declare_dram_parameter("w3p", [P, 4, HT, 256], fp16, False)
    trit = nc.declare_dram_parameter("trit", [2, P, 1024], fp16, False)
    flg = nc.declare_dram_parameter("flg", [P, 8], f32, False)
    vones = nc.declare_dram_parameter("vones", [P, 16, 16], fp16, False)
    ones16 = nc.declare_dram_parameter("ones16", [P, 1], fp16, False)
    out = nc.declare_dram_parameter("out", [NQ, C], f32, True)

    Exp = mybir.ActivationFunctionType.Exp
    Sqrt = mybir.ActivationFunctionType.Sqrt
    Tanh = mybir.ActivationFunctionType.Tanh
    mult = mybir.AluOpType.mult
    add = mybir.AluOpType.add
    DR = mybir.MatmulPerfMode.DoubleRow

    with tile.TileContext(nc, pool_alloc_mode="queue") as tc:
        with tc.tile_pool(name="base", bufs=1) as base:
            h16own = base.tile([P, KC, NQ], fp16)     # rmsnorm(x)^T, own slots
            k8 = base.tile([P, KC, T], f8)            # K^T x16, fp8 (2MB)
            q8 = base.tile([P, KC, NQ], f8)           # Q^T x16, fp8 (1MB)
            v_sb = base.tile([P, 16, 16, 65], fp16)   # V + ones col (4.26MB)
            y16 = base.tile([P, KC, NQ], fp16)        # attn out, feature-major
            acc = base.tile([P, NQ // P, C], fp16)    # x + attn + mlp
            u_sb = base.tile([P, HT, 512], fp16)      # h@w2 then u, per j
            a_sb = base.tile([P, HT, 512], fp16)      # h@w1 staging, per j
            trit_sb = base.tile([P, 2, 1024], fp16)   # diag triangle masks
            flg_sb = base.tile([P, 8], f32)           # ones/zeros pair flags
            ones_sb = base.tile([P, 1], fp16)
            eps_sb = base.tile([1, 1], f32)
            nc.gpsimd.memset(eps_sb[:], EPS)
            nc.sync.dma_start(trit_sb[:], trit.rearrange("m p q -> p m q"))
            nc.sync.dma_start(flg_sb[:], flg[:])
            nc.sync.dma_start(ones_sb[:], ones16[:])
            nc.sync.dma_start(v_sb[:, :, :, 64], vones[:])

            # ---------------- Phase 0+1: rmsnorm, then qkv ----------------
            with tc.tile_pool(name="ph8", bufs=1) as ph8:
                h8 = ph8.tile([P, KC, T], f8)         # rmsnorm(x)^T fp8 (2MB)
                h16oth = ph8.tile([P, KC, NQ], fp16)  # other cores' slots (0,2)
                with tc.tile_pool(name="ph0x", bufs=1) as ph0x, \
                     tc.tile_pool(name="ph0t", bufs=2) as ph0t, \
                     tc.tile_pool(name="ps0", bufs=2, space="PSUM") as ps0:
                    def hsl(tb):
                        dst = h16own if tb % 2 else h16oth
                        c0 = (tb // 2) * 512
                        return dst[:, :, c0:c0 + 512]
                    for tb in range(T // 512):
                        nc.sync.dma_start(hsl(tb), xf[:, tb])
                    with nc.named_scope("rmsnorm"):
                        for tb in range(T // 512):
                            hs = hsl(tb)
                            x2 = ph0x.tile([P, KC, 512], fp16, tag="x2")
                            nc.vector.tensor_tensor(x2[:], hs, hs, mult)
                            ssq = ps0.tile([1, 512], f32, tag="ssq")
                            for kc in range(KC):
                                nc.tensor.matmul(
                                    ssq[:], lhsT=ones_sb[:], rhs=x2[:, kc],
                                    start=(kc == 0), stop=(kc == KC - 1))
                            rms = ph0t.tile([1, 512], f32, tag="rms")
                            nc.scalar.activation(rms[:], ssq[:], Sqrt,
                                                 bias=eps_sb[0:1, :], scale=1.0 / C)
                            rinv = ph0t.tile([1, 512], f32, tag="rinv")
                            nc.vector.reciprocal_approx_fast(rinv[:], rms[:])
                            r16 = ph0t.tile([1, 512], fp16, tag="r16")
                            nc.vector.tensor_copy(r16[:], rinv[:])
                            s_bc = ph0t.tile([P, 512], fp16, tag="sbc")
                            nc.gpsimd.partition_broadcast(s_bc[:], r16[0:1, :])
                            nc.vector.tensor_tensor(
                                hs, hs,
                                s_bc[:, None, :].to_broadcast((P, KC, 512)), mult)
                            nc.vector.tensor_copy(
                                h8[:, :, tb * 512:(tb + 1) * 512], hs)

                # qkv: K/Q in fp8 DoubleRow, V in fp16
                with tc.tile_pool(name="wq8p", bufs=2) as wq8p, \
                     tc.tile_pool(name="wv16p", bufs=1) as wv16p, \
                     tc.tile_pool(name="ps1", bufs=4, space="PSUM") as ps1:
                    with nc.named_scope("qkv"):
                        for half in range(2):
                            wk_c = wq8p.tile([P, KC, 512], f8, tag="wc")
                            nc.sync.dma_start(wk_c[:], wk8[:, half])
                            for oi in range(4):
                                ot = half * 4 + oi
                                for tb in range(4):
                                    ps = ps1.tile([P, 512], f32, tag="mm")
                                    for pr in range(KC // 2):
                                        nc.tensor.matmul(
                                            ps[:],
                                            lhsT=wk_c[:, 2 * pr:2 * pr + 2,
                                                      oi * P:(oi + 1) * P],
                                            rhs=h8[:, 2 * pr:2 * pr + 2,
                                                   tb * 512:(tb + 1) * 512],
                                            start=(pr == 0), stop=(pr == 3),
                                            perf_mode=DR)
                                    nc.vector.tensor_copy(
                                        k8[:, ot, tb * 512:(tb + 1) * 512], ps[:])
                        for half in range(2):
                            wq_c = wq8p.tile([P, KC, 512], f8, tag="wc")
                            nc.sync.dma_start(wq_c[:], wq8[:, half])
                            for oi in range(4):
                                ot = half * 4 + oi
                                for j, t0 in enumerate((512, 1536)):
                                    ps = ps1.tile([P, 512], f32, tag="mm")
                                    for pr in range(KC // 2):
                                        nc.tensor.matmul(
                                            ps[:],
                                            lhsT=wq_c[:, 2 * pr:2 * pr + 2,
                                                      oi * P:(oi + 1) * P],
                                            rhs=h8[:, 2 * pr:2 * pr + 2,
                                                   t0:t0 + 512],
                                            start=(pr == 0), stop=(pr == 3),
                                            perf_mode=DR)
                                    nc.vector.tensor_copy(
                                        q8[:, ot, j * 512:(j + 1) * 512], ps[:])
                        for vf in range(2):
                            wv_c = wv16p.tile([P, KC, 512], fp16, tag="wc")
                            nc.sync.dma_start(wv_c[:], wv[:, vf])
                            for kt in range(16):
                                slot, sub = kt // 4, kt % 4
                                src = h16own if slot % 2 else h16oth
                                c0 = ((slot // 2) * 4 + sub) * P
                                ps = ps1.tile([P, 512], f32, tag="mm")
                                for kc in range(KC):
                                    nc.tensor.matmul(
                                        ps[:], lhsT=src[:, kc, c0:c0 + P],
                                        rhs=wv_c[:, kc, :],
                                        start=(kc == 0), stop=(kc == KC - 1))
                                nc.vector.tensor_copy(
                                    v_sb[:, kt, 8 * vf:8 * (vf + 1), 0:64],
                                    ps[:].rearrange("p (h d) -> p h d", d=64))

            # ---------------- Phase 2: attn + interleaved MLP ----------------
            with tc.tile_pool(name="att", bufs=3) as att, \
                 tc.tile_pool(name="attr", bufs=3) as attr, \
                 tc.tile_pool(name="w12", bufs=2) as w12, \
                 tc.tile_pool(name="w3pool", bufs=1) as w3pool, \
                 tc.tile_pool(name="silu", bufs=2) as silp, \
                 tc.tile_pool(name="ps2s", bufs=2, space="PSUM") as ps2s, \
                 tc.tile_pool(name="ps2y", bufs=2, space="PSUM") as ps2y, \
                 tc.tile_pool(name="psml", bufs=2, space="PSUM") as psml:
                nc.sync.dma_start(acc[:], xq[:])

                # ---- filler generator: mlp_in / mlp_out chunks ----
                # silu is flushed in one burst per j-block so the scalar
                # engine's activation table doesn't thrash between Exp/Swish.
                def filler_gen():
                    # mlp_in for both j-blocks: a/b staged raw to SBUF; silu,
                    # mult and mlp_out run post-attn (keeps the scalar
                    # engine's exp stream free of table switches).
                    for j, t0 in enumerate((0, 512)):
                        tsl = slice(t0, t0 + 512)
                        for ht in range(HT):
                            w1c = w12.tile([P, KC, P], fp16, tag="w1c")
                            w2c = w12.tile([P, KC, P], fp16, tag="w2c")
                            nc.sync.dma_start(w1c[:], w1p[:, ht])
                            nc.sync.dma_start(w2c[:], w2p[:, ht])
                            ps_a = psml.tile([P, 512], f32, tag="mm")
                            for kc in range(KC):
                                nc.tensor.matmul(
                                    ps_a[:], lhsT=w1c[:, kc],
                                    rhs=h16own[:, kc, tsl],
                                    start=(kc == 0), stop=(kc == KC - 1))
                            nc.vector.tensor_copy(a_sb[:, ht, :], ps_a[:])
                            yield
                            ps_b = psml.tile([P, 512], f32, tag="mm")
                            for kc in range(KC):
                                nc.tensor.matmul(
                                    ps_b[:], lhsT=w2c[:, kc],
                                    rhs=h16own[:, kc, tsl],
                                    start=(kc == 0), stop=(kc == KC - 1))
                            nc.vector.tensor_copy(u_sb[:, ht, :], ps_b[:])
                            yield
                        for ht in range(HT):
                            th = silp.tile([P, 512], fp16, tag="th")
                            nc.scalar.activation(th[:], a_sb[:, ht, :], Tanh,
                                                 scale=0.5)
                            sg = silp.tile([P, 512], fp16, tag="sg")
                            nc.vector.tensor_scalar(
                                sg[:], th[:], 1.0, 0.5, add, mult)
                            nc.vector.tensor_tensor(
                                u_sb[:, ht, :], a_sb[:, ht, :],
                                u_sb[:, ht, :], mult)
                            nc.vector.tensor_tensor(
                                u_sb[:, ht, :], sg[:], u_sb[:, ht, :], mult)
                        yield
                        for ofq in range(4):
                            w3c = w3pool.tile([P, HT, 256], fp16, tag="w3c")
                            nc.sync.dma_start(w3c[:], w3p[:, ofq])
                            for qt in range(4):
                                ps = psml.tile([P, 512], f32, tag="mm")
                                for ht in range(HT):
                                    nc.tensor.matmul(
                                        ps[:, 0:256],
                                        lhsT=u_sb[:, ht, qt * P:(qt + 1) * P],
                                        rhs=w3c[:, ht, :],
                                        start=(ht == 0), stop=(ht == HT - 1))
                                asl = acc[:, 4 * j + qt, ofq * 256:(ofq + 1) * 256]
                                nc.vector.tensor_tensor(asl, asl, ps[:, 0:256], add)
                                yield

                fill = filler_gen()
                done = [False]
                head_budget = [99]

                def F(n=1):
                    for _ in range(n):
                        if not done[0] and head_budget[0] > 0:
                            head_budget[0] -= 1
                            try:
                                next(fill)
                            except StopIteration:
                                done[0] = True

                # ---- attention head loop ----
                with nc.named_scope("attn"):
                    for h in range(H):
                        head_budget[0] = 6
                        hp = 64 * (h % 2)
                        ho = h // 2
                        for j, (qofs, npair) in enumerate(((0, 4), (512, 8))):
                            qsl = slice(qofs, qofs + 512)
                            ps_y = ps2y.tile([65, 512], f32, tag="y")
                            p_tiles = {}

                            def emit_scores(pr, j=j, qsl=qsl, hp=hp, ho=ho,
                                            p_tiles=p_tiles):
                                ps_s = ps2s.tile([P, 1024], f32, tag="s")
                                for half in range(2):
                                    kb = pr * 2 + half
                                    nc.tensor.matmul(
                                        ps_s[:, half * 512:(half + 1) * 512],
                                        lhsT=k8[hp:hp + 64, ho, kb * P:(kb + 1) * P],
                                        rhs=q8[hp:hp + 64, ho, qsl],
                                        start=True, stop=True)
                                p_sb = att.tile([P, 1024], fp16, tag="p")
                                nc.scalar.activation(p_sb[:], ps_s[:], Exp,
                                                     scale=1.0 / 2048.0)
                                # masks: first 2 pairs of each unit are
                                # ones-or-zeros (per-core flag), last 2 are the
                                # shared 128-diagonal triangles; unit B pairs
                                # 0-3 are causally full for both core types.
                                if (j == 0 and pr < 2) or (j == 1 and pr in (4, 5)):
                                    nc.vector.tensor_scalar_mul(
                                        p_sb[:], p_sb[:], flg_sb[:, pr:pr + 1])
                                elif (j == 0 and pr >= 2) or (j == 1 and pr >= 6):
                                    m = pr - 2 if j == 0 else pr - 6
                                    nc.vector.tensor_tensor(
                                        p_sb[:], p_sb[:], trit_sb[:, m, :], mult)
                                p_tiles[pr] = p_sb

                            def emit_av(pr, npair=npair, h=h, ps_y=ps_y,
                                        p_tiles=p_tiles):
                                p_sb = p_tiles.pop(pr)
                                for half in range(2):
                                    kb = pr * 2 + half
                                    nc.tensor.matmul(
                                        ps_y[:],
                                        lhsT=v_sb[:, kb, h, :],
                                        rhs=p_sb[:, half * 512:(half + 1) * 512],
                                        start=(kb == 0), stop=(kb == 2 * npair - 1))

                            for pr in range(npair):
                                emit_scores(pr)
                                F(1)
                                if pr >= 1:
                                    emit_av(pr - 1)
                            emit_av(npair - 1)
                            yu = attr.tile([64, 512], fp16, tag="yu")
                            nc.vector.tensor_copy(yu[:], ps_y[0:64, :])
                            den = attr.tile([1, 512], f32, tag="den")
                            nc.vector.tensor_copy(den[:], ps_y[64:65, :])
                            rinv = attr.tile([1, 512], f32, tag="rd")
                            nc.vector.reciprocal_approx_fast(rinv[:], den[:])
                            r_bc = attr.tile([64, 512], f32, tag="rbc")
                            nc.gpsimd.partition_broadcast(r_bc[:], rinv[0:1, :])
                            nc.vector.tensor_tensor(
                                y16[hp:hp + 64, ho, j * 512:(j + 1) * 512],
                                yu[:], r_bc[:], mult)
                            F(2)

                # drain remaining filler work
                head_budget[0] = 999
                F(200)

            # ---- proj (attn pools closed; own scope) ----
            with tc.tile_pool(name="projw", bufs=2) as projw, \
                 tc.tile_pool(name="ps3", bufs=2, space="PSUM") as ps3:
                with nc.named_scope("proj"):
                    for of in range(2):
                        wp_c = projw.tile([P, KC, 512], fp16, tag="wpc")
                        nc.sync.dma_start(wp_c[:], wp[:, of])
                        for qt in range(NQ // P):
                            ps = ps3.tile([P, 512], f32, tag="mm")
                            for kc in range(KC):
                                nc.tensor.matmul(
                                    ps[:],
                                    lhsT=y16[:, kc, qt * P:(qt + 1) * P],
                                    rhs=wp_c[:, kc, :],
                                    start=(kc == 0), stop=(kc == KC - 1))
                            asl = acc[:, qt, of * 512:(of + 1) * 512]
                            nc.vector.tensor_tensor(asl, asl, ps[:], add)

            # ---- fp16 acc -> f32 out ----
            with tc.tile_pool(name="stg", bufs=2) as stg:
                outv = out.rearrange("(qt p) f -> p qt f", p=P)
                for qt in range(NQ // P):
                    st = stg.tile([P, C], f32, tag="st")
                    nc.vector.tensor_copy(st[:], acc[:, qt, :])
                    nc.sync.dma_start(outv[:, qt, :], st[:])

    nc.finalize()
    _NC_CACHE["nc"] = nc
    return nc


def _prep_inputs(x, w_attn, w_proj, w1, w2, w3, g1, g2):
    """Host-side preprocessing -> list of 8 per-core input maps."""
    x = np.asarray(x, np.float32)
    w_attn = np.asarray(w_attn, np.float32)
    g1 = np.asarray(g1, np.float32)
    g2 = np.asarray(g2, np.float32)

    def to8(w):
        return np.clip(w * WS, -240, 240).astype(E4NP)

    def whalf(w):
        # [C, C] -> [P, half, KC, 512] contiguous per-partition DMA layout
        return np.ascontiguousarray(
            w.reshape(KC, P, 2, 512).transpose(1, 2, 0, 3))

    wq8 = whalf(to8(g1[:, None] * w_attn[:, 0:C]))
    wk8 = whalf(to8(g1[:, None] * w_attn[:, C:2 * C]))
    wvh = whalf((g1[:, None] * w_attn[:, 2 * C:3 * C]).astype(np.float16))
    wph = whalf(np.asarray(w_proj, np.float32).astype(np.float16))
    w1p = np.zeros((C, HIDP), np.float16)
    w1p[:, :HID] = (g2[:, None] * np.asarray(w1, np.float32)).astype(np.float16)
    w2p = np.zeros((C, HIDP), np.float16)
    w2p[:, :HID] = (g2[:, None] * np.asarray(w2, np.float32)).astype(np.float16)
    # [C, HIDP] -> [P, HT, KC, P]
    w1p = np.ascontiguousarray(w1p.reshape(KC, P, HT, P).transpose(1, 2, 0, 3))
    w2p = np.ascontiguousarray(w2p.reshape(KC, P, HT, P).transpose(1, 2, 0, 3))
    w3f = np.zeros((HIDP, C), np.float16)
    w3f[:HID, :] = np.asarray(w3, np.float32).astype(np.float16)
    # [HIDP, C] -> [P, 4, HT, 256]
    w3p = np.ascontiguousarray(w3f.reshape(HT, P, 4, 256).transpose(1, 2, 0, 3))

    # triangle masks: tri[m][i, q] = 1 if i + 128*m <= q  (q in 0..511);
    # trit entry m = [tri(2m) | tri(2m+1)] covering one kb pair.
    ii = np.arange(P)[:, None]
    qq = np.arange(512)[None, :]
    tri = [(ii + P * m <= qq).astype(np.float16) for m in range(4)]
    trit = np.stack([np.concatenate([tri[0], tri[1]], axis=1),
                     np.concatenate([tri[2], tri[3]], axis=1)], axis=0)

    vones = np.ones((P, 16, 16), np.float16)
    ones16 = np.ones((P, 1), np.float16)

    perms = {0: [1, 0, 2, 3], 1: [0, 1, 3, 2]}

    in_maps = []
    for core in range(8):
        b, h = core // 2, core % 2
        pi = perms[h]
        xp = np.concatenate([x[b, g * 512:(g + 1) * 512] for g in pi], axis=0)
        # x^T [C, T] -> [P, tb, KC, 512]
        xt = xp.T.astype(np.float16).reshape(KC, P, 4, 512)
        xf = np.ascontiguousarray(xt.transpose(1, 2, 0, 3))
        xqr = np.concatenate([xp[512:1024], xp[1536:2048]], axis=0).astype(np.float16)
        xq = np.ascontiguousarray(xqr.reshape(8, P, C).transpose(1, 0, 2))

        # pair flags: unit A (q block pi[1]) pairs 0-1 cover k slot 0
        # (block pi[0]); unit B (q block pi[3]) pairs 4-5 cover k slot 2
        # (block pi[2]). 1.0 = causally full, 0.0 = fully masked.
        flg = np.ones((P, 8), np.float32)
        flg[:, 0] = flg[:, 1] = 1.0 if pi[0] < pi[1] else 0.0
        flg[:, 4] = flg[:, 5] = 1.0 if pi[2] < pi[3] else 0.0

        in_maps.append({
            "xf": xf, "xq": xq, "wq8": wq8, "wk8": wk8, "wv": wvh, "wp": wph,
            "w1p": w1p, "w2p": w2p, "w3p": w3p, "trit": trit, "flg": flg,
            "vones": vones, "ones16": ones16,
        })
    return in_maps


def _run(inputs, trace=False):
    nc = _build()
    in_maps = _prep_inputs(**inputs)
    res = run_bass_kernel_spmd(
        nc, in_maps, core_ids=list(range(8)), trace=trace,
        trace_cores=list(range(8)) if trace else None)
    B = 4
    perms = {0: [1, 0, 2, 3], 1: [0, 1, 3, 2]}
    out = np.empty((B, T, C), np.float32)
    for core in range(8):
        b, h = core // 2, core % 2
        pi = perms[h]
        r = res.results[core]["out"]
        out[b, pi[1] * 512:(pi[1] + 1) * 512] = r[0:512]
        out[b, pi[3] * 512:(pi[3] + 1) * 512] = r[512:1024]
    return out, res


def kernel(**inputs):
    out, _ = _run(inputs, trace=False)
    return out
